# revision 17
# baseline (speedup 1.0000x reference)
"""Trainium2 Bass kernel for CondGIN (3-layer GIN + graph pooling + cond MLP head).

Strategy (8 NeuronCores, SPMD single NEFF):
  - Graphs are assigned to cores (32 graphs/core, edge-balanced). A core owns its
    graphs' nodes and all edges whose dst lands in them.
  - Node features live in a replicated DRAM "table" [TBL, 128] (256B bf16 rows).
    For layers 1-2, each core dma_gather's h[src] for its edges (merged into
    5-block super-gathers to amortize SWDGE overhead), aggregates per-dst with
    PE matmuls against DVE-built one-hot matrices (exact f32 PSUM, no scatter
    races), runs the GIN MLP feature-major (BN folded into W2/b2 on host),
    writes its slice, and an 8-core AllGather refreshes the table.
  - Layer 0's sources are known on the host (h = x), so they are pre-expanded
    into the gather-slot layout and streamed with sequential DMA — no
    descriptor generation (saves ~1ms of gpsimd SWDGE time).
  - The self contribution h_i is NOT gathered: each block's previous-layer hT
    stays in SBUF (xT / hstoreA / hstoreB ping-pong) and is added into aT
    directly, cutting ~1 gather chunk per block side.
  - Gather indices are int16 (HW limit 32767): the table is addressed via a lo
    base and a hi base aligned to a core boundary; the host block packer
    balances per-block lo/hi edge counts separately so the global chunk counts
    CPB_LO/CPB_HI stay minimal (padding ~2%).
  - Pooling (segment_sum over batch) is a matmul against per-block graph
    one-hots accumulated in PSUM; the tiny cond MLP + FC head run per-core on
    its 32 graphs.

Measured on 8 axon trn2 cores: 2.49 ms HW exec, rel err 2.6e-4 (baseline
dma_gather-everything version: 3.52 ms). Remaining wall: SWDGE descriptor
generation at ~7.75 ns/row on gpsimd for layers 1-2 (~0.8 ms/layer).
"""

import math
import os
from contextlib import ExitStack

import numpy as np

import concourse.bass as bass
import concourse.bacc as bacc
import concourse.mybir as mybir
import concourse.tile as tile
from concourse.bass_utils import run_bass_kernel_spmd

F32 = mybir.dt.float32
BF16 = mybir.dt.bfloat16
I16 = mybir.dt.int16

D = 96          # feature dim
DP = 128        # padded row width (elements)
BN_EPS = 1e-5
LRELU_ALPHA = 0.2


# ----------------------------------------------------------------------------
# Host-side layout construction
# ----------------------------------------------------------------------------

class Layout:
    pass


def build_layout(edge_index, batch, n_graphs, n_cores=8, lo_split=32768,
                 use_bf16=False, block_slack=1):
    """Compute the full static layout + per-core index data from the inputs."""
    lay = Layout()
    src = np.asarray(edge_index[0], dtype=np.int64)
    dst = np.asarray(edge_index[1], dtype=np.int64)
    batch = np.asarray(batch, dtype=np.int64)
    N = batch.shape[0]
    E = src.shape[0]
    G = n_graphs
    lay.n_cores = n_cores
    lay.use_bf16 = use_bf16
    assert G % n_cores == 0
    GPC = G // n_cores
    lay.GPC = GPC

    # graph node ranges (batch is sorted)
    gstart = np.searchsorted(batch, np.arange(G + 1))
    gsize = np.diff(gstart)

    # per-graph edge counts (by dst's graph)
    dst_graph = np.searchsorted(gstart, dst, side="right") - 1
    gedges = np.bincount(dst_graph, minlength=G)

    # graphs -> cores: balanced LPT on (edges + nodes), exactly GPC per core
    order = np.argsort(-(gedges + gsize))
    core_load = np.zeros(n_cores, dtype=np.int64)
    core_cnt = np.zeros(n_cores, dtype=np.int64)
    graph_core = np.zeros(G, dtype=np.int64)
    for g in order:
        open_cores = np.nonzero(core_cnt < GPC)[0]
        c = open_cores[np.argmin(core_load[open_cores])]
        graph_core[g] = c
        core_load[c] += gedges[g] + gsize[g]
        core_cnt[c] += 1
    graph_lists = [np.nonzero(graph_core == c)[0] for c in range(n_cores)]
    lay.graph_lists = graph_lists

    node_core = graph_core[batch]  # core of each node
    # per-node in-degree (real edges)
    indeg = np.bincount(dst, minlength=N)

    # nodes per core
    core_nodes = [np.nonzero(node_core == c)[0] for c in range(n_cores)]
    max_n = max(len(x) for x in core_nodes)
    B = int(math.ceil(max_n / 128.0)) + block_slack
    lay.B = B
    S = B * 128
    lay.S = S
    TBL = 2 + n_cores * S
    lay.TBL = TBL
    # align the int16 lo/hi base split to a core boundary so a node's
    # lo/hi-ness is known before block packing (cores 0..SPLIT_CORE-1 are lo)
    split_core = min((lo_split - 1) // S, n_cores)
    lo_split = 1 + split_core * S
    assert TBL - lo_split <= 32768, (TBL, lo_split)
    assert lo_split <= 32768
    lay.lo_split = lo_split

    # per-node in-degree split by source row half (src core < split_core)
    src_is_lo = node_core[src] < split_core
    indeg_lo = np.bincount(dst[src_is_lo], minlength=N)
    indeg_hi = indeg - indeg_lo

    # --- assign nodes to (block, pos) per core, balancing lo/hi edge counts
    # separately (the gather chunk counts CPB_LO/CPB_HI are global maxima of
    # per-block lo/hi loads, so both must be flat).
    node_block = np.full(N, -1, dtype=np.int64)
    node_pos = np.full(N, -1, dtype=np.int64)
    for c in range(n_cores):
        nodes = core_nodes[c]
        dlo = indeg_lo[nodes]
        dhi = indeg_hi[nodes]
        order = np.argsort(-(dlo + dhi))
        lload = np.zeros(B, dtype=np.int64)
        hload = np.zeros(B, dtype=np.int64)
        bcnt = np.zeros(B, dtype=np.int64)
        avg_l = max(dlo.sum() / B, 1.0)
        avg_h = max(dhi.sum() / B, 1.0)
        for i in order:
            open_b = np.nonzero(bcnt < 128)[0]
            score = np.maximum((lload[open_b] + dlo[i]) / avg_l,
                               (hload[open_b] + dhi[i]) / avg_h)
            b = open_b[np.argmin(score)]
            node_block[nodes[i]] = b
            node_pos[nodes[i]] = bcnt[b]
            lload[b] += dlo[i]
            hload[b] += dhi[i]
            bcnt[b] += 1

    # global table row of each node
    node_row = 1 + node_core * S + node_block * 128 + node_pos
    lay.node_row = node_row

    # --- per (core, block): build edge lists (incl self edges), split lo/hi ---
    # edge entry: (src_row, pos_of_dst)
    all_src_row = node_row[src]
    all_dst_core = node_core[dst]
    all_dst_block = node_block[dst]
    all_dst_pos = node_pos[dst]

    # chunk counts
    lo_cnt = np.zeros((n_cores, B), dtype=np.int64)
    hi_cnt = np.zeros((n_cores, B), dtype=np.int64)
    # bucket edges per (core, block)
    edge_lists = [[None] * B for _ in range(n_cores)]
    key = all_dst_core * B + all_dst_block
    order = np.argsort(key, kind="stable")
    bounds = np.searchsorted(key[order], np.arange(n_cores * B + 1))
    for c in range(n_cores):
        for b in range(B):
            k = c * B + b
            sel = order[bounds[k]:bounds[k + 1]]
            srcs = all_src_row[sel]
            poss = all_dst_pos[sel]
            is_lo = srcs < lo_split
            edge_lists[c][b] = (srcs, poss, is_lo)
            lo_cnt[c, b] = int(is_lo.sum())
            hi_cnt[c, b] = int((~is_lo).sum())

    CPB_LO = int(math.ceil(lo_cnt.max() / 128.0)) if lo_cnt.max() > 0 else 0
    CPB_HI = int(math.ceil(hi_cnt.max() / 128.0)) if hi_cnt.max() > 0 else 0
    CPB_LO = max(CPB_LO, 1)
    lay.CPB_LO, lay.CPB_HI = CPB_LO, CPB_HI
    CPB = CPB_LO + CPB_HI
    lay.CPB = CPB

    zero_lo = 0                      # table row 0 is a zero row
    zero_hi = TBL - 1 - lo_split     # last row is a zero row (hi-relative)

    # --- emit per-core tensors ---
    # absolute source-row ids per gather slot (for host-side l0 pre-expansion)
    slots_lo = np.zeros((n_cores, B * CPB_LO * 128), dtype=np.int64)
    slots_hi = np.full((n_cores, max(B * CPB_HI * 128, 1)), TBL - 1,
                       dtype=np.int64)
    idx_lo = np.zeros((n_cores, 128, B * CPB_LO * 8), dtype=np.int16)
    idx_hi = np.zeros((n_cores, 128, max(B * CPB_HI * 8, 1)), dtype=np.int16)
    fdt = np.float32
    dstpos = np.full((n_cores, 128, B * CPB), -1.0, dtype=fdt)
    graphpos = np.full((n_cores, 128, B), -1.0, dtype=fdt)

    def fill_idx(arr, c, col0, vals):
        # vals: int array of length n; slot i -> group row i%16, col col0+i//16,
        # replicated across the 8 groups of 16 partitions
        n = len(vals)
        cols = int(math.ceil(n / 16.0))
        buf = np.zeros(16 * cols, dtype=np.int16)
        buf[:n] = vals.astype(np.int16)
        buf = buf.reshape(cols, 16).T  # [16, cols]
        for g in range(8):
            arr[c, g * 16:(g + 1) * 16, col0:col0 + cols] = buf

    # gather-merge group size: one dma_gather instruction covers GB blocks
    GB = 5 if B % 5 == 0 else (2 if B % 2 == 0 else 1)
    lay.GB = GB

    for c in range(n_cores):
        vv_acc = {0: [], 1: []}
        for b in range(B):
            srcs, poss, is_lo = edge_lists[c][b]
            for half, (cap, colw, arr, zrow, base) in enumerate([
                (CPB_LO, CPB_LO * 8, idx_lo, zero_lo, 0),
                (CPB_HI, CPB_HI * 8, idx_hi, zero_hi, lo_split),
            ]):
                if cap == 0:
                    continue
                m = is_lo if half == 0 else ~is_lo
                v = srcs[m] - base
                p = poss[m]
                n = len(v)
                assert n <= cap * 128, (c, b, half, n, cap * 128)
                vv = np.full(cap * 128, zrow, dtype=np.int64)
                vv[:n] = v
                vv_acc[half].append(vv)
                slots = slots_lo if half == 0 else slots_hi
                slots[c, b * cap * 128:(b + 1) * cap * 128] = vv + base
                # dstpos: chunk col index within the block = half offset + chunk
                pp = np.full(cap * 128, -1.0, dtype=fdt)
                pp[:n] = p.astype(fdt)
                coff = b * CPB + (0 if half == 0 else CPB_LO)
                dstpos[c, :, coff:coff + cap] = pp.reshape(cap, 128).T
            if (b + 1) % GB == 0:
                # emit the group's concatenated slot lists, 16-wrapped per
                # gather instruction (not per block)
                g0 = b + 1 - GB
                for half, (cap, colw, arr) in enumerate([
                    (CPB_LO, CPB_LO * 8, idx_lo),
                    (CPB_HI, CPB_HI * 8, idx_hi),
                ]):
                    if cap == 0 or not vv_acc[half]:
                        continue
                    fill_idx(arr, c, g0 * colw,
                             np.concatenate(vv_acc[half]))
                vv_acc = {0: [], 1: []}

        # graphpos: slot -> ordinal of its graph within this core's graph list
        gl = graph_lists[c]
        gmap = {g: j for j, g in enumerate(gl)}
        nodes = core_nodes[c]
        for nid in nodes:
            b = node_block[nid]
            p = node_pos[nid]
            graphpos[c, p, b] = float(gmap[batch[nid]])

    lay.idx_lo, lay.idx_hi = idx_lo, idx_hi
    lay.slots_lo, lay.slots_hi = slots_lo, slots_hi
    lay.dstpos, lay.graphpos = dstpos, graphpos
    return lay


def build_x_table(lay, x):
    tdt = np.float32 if not lay.use_bf16 else np.dtype("bfloat16")
    try:
        tdt = np.dtype(tdt)
        tbl = np.zeros((lay.TBL, DP), dtype=tdt)
    except TypeError:
        import ml_dtypes
        tdt = np.dtype(ml_dtypes.bfloat16)
        tbl = np.zeros((lay.TBL, DP), dtype=tdt)
    tbl[lay.node_row, :D] = x.astype(tdt)
    return tbl


def fold_weights(inputs):
    """Fold BN (eval) affine transforms into adjacent linear layers. Returns a
    dict of device-ready weight arrays (float32)."""
    f = {k: np.asarray(v, dtype=np.float64) for k, v in inputs.items()
         if k not in ("x", "cond", "edge_index", "batch")}
    out = {}
    L = f["conv_W1"].shape[0]
    for layer in range(L):
        s = f["conv_g"][layer] / np.sqrt(f["conv_var"][layer] + BN_EPS)
        t = f["conv_beta"][layer] - f["conv_mean"][layer] * s
        W2p = s[:, None] * f["conv_W2"][layer]
        b2p = t @ f["conv_W2"][layer] + f["conv_b2"][layer]
        out[f"W1_{layer}"] = f["conv_W1"][layer].astype(np.float32)          # [96,96] lhsT
        out[f"b1_{layer}"] = f["conv_b1"][layer].astype(np.float32)[:, None]  # [96,1]
        out[f"W2_{layer}"] = W2p.astype(np.float32)
        out[f"b2_{layer}"] = b2p.astype(np.float32)[:, None]
    # cond MLP: bn1 folds into cW1/cb1
    s = f["cg"] / np.sqrt(f["cvar"] + BN_EPS)
    t = f["cbeta"] - f["cmean"] * s
    out["cW1"] = (f["cW1"] * s[None, :]).astype(np.float32)       # [7,5]
    out["cb1"] = ((f["cb1"] * s) + t).astype(np.float32)[:, None]  # [5,1]
    out["cW2"] = f["cW2"].astype(np.float32)                      # [5,5]
    out["cb2"] = f["cb2"].astype(np.float32)[:, None]             # [5,1]
    # final bn folds into fc
    s = f["bn_g"] / np.sqrt(f["bn_var"] + BN_EPS)
    t = f["bn_b"] - f["bn_mean"] * s
    fcW = s[:, None] * f["fc_W"]
    fcb = t @ f["fc_W"] + f["fc_b"]
    CH = f["cW2"].shape[1]
    out["fcWc"] = fcW[:CH].astype(np.float32)        # [5,64]
    out["fcWd"] = fcW[CH:].astype(np.float32)        # [96,64]
    out["fcb"] = fcb.astype(np.float32)[:, None]     # [64,1]
    return out


# ----------------------------------------------------------------------------
# Bass kernel builder
# ----------------------------------------------------------------------------

def build_bass(lay, n_layers=3, lat=64):
    n_cores = lay.n_cores
    B, CPB_LO, CPB_HI, CPB = lay.B, lay.CPB_LO, lay.CPB_HI, lay.CPB
    S, TBL, GPC = lay.S, lay.TBL, lay.GPC
    LO = lay.lo_split
    tdt = BF16 if lay.use_bf16 else F32
    CD = 7
    CH = 5

    nc = bacc.Bacc("TRN2", target_bir_lowering=False, debug=False,
                   num_devices=n_cores)

    # ---- I/O ----
    # layer-0 sources pre-expanded on host into gather-slot layout
    exp0_lo = nc.dram_tensor("exp0_lo", [B * CPB_LO, 128, DP], tdt,
                             kind="ExternalInput")
    exp0_hi = nc.dram_tensor("exp0_hi", [max(B * CPB_HI, 1), 128, DP], tdt,
                             kind="ExternalInput")
    xT = nc.dram_tensor("xT", [D, S], tdt, kind="ExternalInput")
    idx_lo = nc.dram_tensor("idx_lo", [128, B * CPB_LO * 8], I16, kind="ExternalInput")
    idx_hi = nc.dram_tensor("idx_hi", [128, max(B * CPB_HI * 8, 1)], I16, kind="ExternalInput")
    dstpos = nc.dram_tensor("dstpos", [128, B * CPB], tdt, kind="ExternalInput")
    graphpos = nc.dram_tensor("graphpos", [128, B], tdt, kind="ExternalInput")
    iota128 = nc.dram_tensor("iota128", [128, 128], tdt, kind="ExternalInput")
    iotaG = nc.dram_tensor("iotaG", [128, GPC], tdt, kind="ExternalInput")
    identity = nc.dram_tensor("identity", [128, 128], F32, kind="ExternalInput")
    condT = nc.dram_tensor("condT", [CD, GPC], F32, kind="ExternalInput")
    wnames = []
    for l in range(n_layers):
        wnames += [(f"W1_{l}", [D, D]), (f"b1_{l}", [D, 1]),
                   (f"W2_{l}", [D, D]), (f"b2_{l}", [D, 1])]
    wnames += [("cW1", [CD, CH]), ("cb1", [CH, 1]), ("cW2", [CH, CH]),
               ("cb2", [CH, 1]), ("fcWc", [CH, lat]), ("fcWd", [D, lat]),
               ("fcb", [lat, 1])]
    wt_dram = {nm: nc.dram_tensor(nm, shp, F32, kind="ExternalInput")
               for nm, shp in wnames}
    outT = nc.dram_tensor("outT", [lat, GPC], F32, kind="ExternalOutput")

    with ExitStack() as stack:
        tc = stack.enter_context(tile.TileContext(nc))

        dram = stack.enter_context(tc.tile_pool(name="dram", bufs=1, space="DRAM"))
        table_int = dram.tile([TBL, DP], tdt)
        my_slice = dram.tile([S, DP], tdt)

        const = stack.enter_context(tc.tile_pool(name="const", bufs=1))
        sb = {}
        for nm, shp in wnames:
            sb[nm] = const.tile(shp, F32, name=f"sb_{nm}")
            nc.sync.dma_start(sb[nm], wt_dram[nm].ap())
        idx_lo_sb = const.tile([128, B * CPB_LO * 8], I16, name="idx_lo_sb")
        nc.sync.dma_start(idx_lo_sb, idx_lo.ap())
        if CPB_HI > 0:
            idx_hi_sb = const.tile([128, B * CPB_HI * 8], I16, name="idx_hi_sb")
            nc.sync.dma_start(idx_hi_sb, idx_hi.ap())
        dstpos_sb = const.tile([128, B * CPB], tdt, name="dstpos_sb")
        nc.sync.dma_start(dstpos_sb, dstpos.ap())
        graphpos_sb = const.tile([128, B], tdt, name="graphpos_sb")
        nc.sync.dma_start(graphpos_sb, graphpos.ap())
        iota128_sb = const.tile([128, 128], tdt, name="iota128_sb")
        nc.sync.dma_start(iota128_sb, iota128.ap())
        iotaG_sb = const.tile([128, GPC], tdt, name="iotaG_sb")
        nc.sync.dma_start(iotaG_sb, iotaG.ap())
        ident_sb = const.tile([128, 128], F32, name="ident_sb")
        nc.sync.dma_start(ident_sb, identity.ap())
        condT_sb = const.tile([CD, GPC], F32, name="condT_sb")
        nc.sync.dma_start(condT_sb, condT.ap())
        xT_sb = const.tile([D, S], tdt, name="xT_sb")
        nc.sync.dma_start(xT_sb, xT.ap())
        hstoreA = const.tile([D, S], tdt, name="hstoreA")
        hstoreB = const.tile([D, S], tdt, name="hstoreB")
        zero_sb = const.tile([1, DP], tdt, name="zero_sb")
        nc.vector.memset(zero_sb, 0.0)
        # zero rows of the internal table (rows 0 and TBL-1; AG writes 1..TBL-2)
        nc.sync.dma_start(table_int[0:1, :], zero_sb)
        nc.sync.dma_start(table_int[TBL - 1:TBL, :], zero_sb)

        # pools
        glo_p = stack.enter_context(tc.tile_pool(name="glo", bufs=3))
        ghi_p = stack.enter_context(tc.tile_pool(name="ghi", bufs=3))
        oh_p = stack.enter_context(tc.tile_pool(name="oh", bufs=3))
        mlp_p = stack.enter_context(tc.tile_pool(name="mlp", bufs=3))
        rows_p = stack.enter_context(tc.tile_pool(name="rows", bufs=3))
        psa_p = stack.enter_context(tc.tile_pool(name="psa", bufs=2, space="PSUM"))
        psm_p = stack.enter_context(tc.tile_pool(name="psm", bufs=4, space="PSUM"))
        psp_p = stack.enter_context(tc.tile_pool(name="psp", bufs=1, space="PSUM"))

        pooled_ps = None
        nreg_lo = CPB_LO * 128
        nreg_hi = CPB_HI * 128

        GB = lay.GB
        NGRP = B // GB
        nsup_lo = GB * CPB_LO * 128
        nsup_hi = GB * CPB_HI * 128
        lo_ap = table_int[0:LO, :]
        hi_ap = table_int[LO:TBL, :] if TBL > LO else None

        for l in range(n_layers):
            last = l == n_layers - 1
            if last:
                pooled_ps = psp_p.tile([D, GPC], F32, name="pooled_ps")

            for grp in range(NGRP):
              b0 = grp * GB
              if True:
                glo = glo_p.tile([128, GB * CPB_LO, DP], tdt, name="glo", tag="glo")
                if l == 0:
                    nc.sync.dma_start(
                        glo,
                        exp0_lo.ap()[b0 * CPB_LO:(b0 + GB) * CPB_LO]
                        .rearrange("c p e -> p c e"))
                else:
                    nc.gpsimd.dma_gather(
                        glo, lo_ap,
                        idx_lo_sb[:, b0 * CPB_LO * 8:(b0 + GB) * CPB_LO * 8],
                        nsup_lo, nsup_lo, DP, single_packet=False)
                if CPB_HI > 0:
                    ghi = ghi_p.tile([128, GB * CPB_HI, DP], tdt, name="ghi", tag="ghi")
                    if l == 0:
                        nc.sync.dma_start(
                            ghi,
                            exp0_hi.ap()[b0 * CPB_HI:(b0 + GB) * CPB_HI]
                            .rearrange("c p e -> p c e"))
                    else:
                        nc.gpsimd.dma_gather(
                            ghi, hi_ap,
                            idx_hi_sb[:, b0 * CPB_HI * 8:(b0 + GB) * CPB_HI * 8],
                            nsup_hi, nsup_hi, DP, single_packet=False)

              for bb in range(GB):
                b = b0 + bb
                # one-hot for all chunks of this block: [128, CPB, 128]
                oh = oh_p.tile([128, CPB, 128], tdt, name="oh", tag="oh")
                iota_b = bass.AP(iota128_sb.tensor, iota128_sb.offset,
                                 [iota128_sb.ap[0], [0, CPB], [1, 128]])
                dp_b = dstpos_sb[:, b * CPB:(b + 1) * CPB]
                dp_bb = bass.AP(dp_b.tensor, dp_b.offset,
                                [dp_b.ap[0], [1, CPB], [0, 128]])
                nc.vector.tensor_tensor(out=oh, in0=dp_bb, in1=iota_b,
                                        op=mybir.AluOpType.is_equal)

                # aggregate: psum[dp_feat, pos] += gathered.T @ onehot
                ps_a = psa_p.tile([DP, 128], F32, name="ps_a", tag="ps_a")
                for c in range(CPB):
                    g = (glo[:, bb * CPB_LO + c] if c < CPB_LO
                         else ghi[:, bb * CPB_HI + (c - CPB_LO)])
                    nc.tensor.matmul(ps_a, g, oh[:, c],
                                     start=(c == 0), stop=(c == CPB - 1))

                # MLP (feature-major); add h_prev (self edge done locally)
                src_h = [xT_sb, hstoreA, hstoreB][l]
                aT = mlp_p.tile([D, 128], F32, name="aT", tag="aT")
                nc.vector.tensor_tensor(out=aT, in0=ps_a[0:D, :],
                                        in1=src_h[:, b * 128:(b + 1) * 128],
                                        op=mybir.AluOpType.add)
                def lrelu_bias(out_tile, ps, bias_ap, tagpfx):
                    y = mlp_p.tile([D, 128], F32, name=f"{tagpfx}y",
                                   tag=f"{tagpfx}y")
                    t = mlp_p.tile([D, 128], F32, name=f"{tagpfx}t",
                                   tag=f"{tagpfx}t")
                    nc.vector.tensor_scalar_add(y, ps, bias_ap)
                    nc.vector.tensor_scalar(t, ps, bias_ap, LRELU_ALPHA,
                                            op0=mybir.AluOpType.add,
                                            op1=mybir.AluOpType.mult)
                    nc.vector.tensor_tensor(out=out_tile, in0=y, in1=t,
                                            op=mybir.AluOpType.max)

                ps1 = psm_p.tile([D, 128], F32, name="ps1", tag="psm")
                nc.tensor.matmul(ps1, sb[f"W1_{l}"], aT, start=True, stop=True)
                u = mlp_p.tile([D, 128], F32, name="u", tag="u")
                lrelu_bias(u, ps1, sb[f"b1_{l}"], "u")
                ps2 = psm_p.tile([D, 128], F32, name="ps2", tag="psm")
                nc.tensor.matmul(ps2, sb[f"W2_{l}"], u, start=True, stop=True)
                hT = mlp_p.tile([D, 128], F32, name="hT", tag="hT")
                lrelu_bias(hT, ps2, sb[f"b2_{l}"], "h")
                if not last:
                    dst_h = [hstoreA, hstoreB, None][l]
                    nc.vector.tensor_copy(dst_h[:, b * 128:(b + 1) * 128], hT)
                # transpose to rows
                ps3 = psm_p.tile([128, D], F32, name="ps3", tag="psm")
                nc.tensor.transpose(ps3, hT, ident_sb[0:D, 0:D])
                hrows = rows_p.tile([128, DP], tdt, name="hrows", tag="hrows")
                nc.vector.tensor_copy(hrows[:, 0:D], ps3)
                if not last:
                    nc.vector.memset(hrows[:, D:DP], 0.0)
                    nc.sync.dma_start(my_slice[b * 128:(b + 1) * 128, :], hrows)
                else:
                    ohg = mlp_p.tile([128, GPC], tdt, name="ohg", tag="ohg")
                    gp_b = graphpos_sb[:, b:b + 1]
                    gp_bb = bass.AP(gp_b.tensor, gp_b.offset,
                                    [gp_b.ap[0], [0, GPC]])
                    nc.vector.tensor_tensor(out=ohg, in0=iotaG_sb, in1=gp_bb,
                                            op=mybir.AluOpType.is_equal)
                    nc.tensor.matmul(pooled_ps, hrows[:, 0:D], ohg,
                                     start=(b == 0), stop=(b == B - 1),
                                     skip_group_check=True)

            if not last:
                nc.gpsimd.collective_compute(
                    "AllGather", mybir.AluOpType.bypass,
                    replica_groups=[list(range(n_cores))],
                    ins=[my_slice.opt()],
                    outs=[table_int[1:TBL - 1, :].opt()],
                )

        # ---- head ----
        pooled_sb = const.tile([D, GPC], F32, name="pooled_sb")
        nc.vector.tensor_copy(pooled_sb, pooled_ps)
        psc = psm_p.tile([CH, GPC], F32, name="psc", tag="psm")
        nc.tensor.matmul(psc, sb["cW1"], condT_sb, start=True, stop=True)
        c1 = const.tile([CH, GPC], F32, name="c1")
        nc.scalar.activation(c1, psc, mybir.ActivationFunctionType.Relu,
                             bias=sb["cb1"], scale=1.0)
        psc2 = psm_p.tile([CH, GPC], F32, name="psc2", tag="psm")
        nc.tensor.matmul(psc2, sb["cW2"], c1, start=True, stop=True)
        c2 = const.tile([CH, GPC], F32, name="c2")
        nc.scalar.activation(c2, psc2, mybir.ActivationFunctionType.Relu,
                             bias=sb["cb2"], scale=1.0)
        pso = psm_p.tile([lat, GPC], F32, name="pso", tag="psm")
        nc.tensor.matmul(pso, sb["fcWc"], c2, start=True, stop=False)
        nc.tensor.matmul(pso, sb["fcWd"], pooled_sb, start=False, stop=True)
        out_sb = const.tile([lat, GPC], F32, name="out_sb")
        nc.vector.tensor_scalar_add(out_sb, pso, sb["fcb"])
        nc.sync.dma_start(outT.ap(), out_sb)

    nc.compile()
    return nc


# ----------------------------------------------------------------------------
# Entry point
# ----------------------------------------------------------------------------

def _np_bf16():
    import ml_dtypes
    return np.dtype(ml_dtypes.bfloat16)


def make_in_maps(lay, inputs, n_layers=3, lat=64):
    x = np.asarray(inputs["x"], dtype=np.float32)
    cond = np.asarray(inputs["cond"], dtype=np.float32)
    tdt = _np_bf16() if lay.use_bf16 else np.float32
    wt = fold_weights(inputs)
    x_table = np.zeros((lay.TBL, DP), dtype=tdt)
    x_table[lay.node_row, :D] = x.astype(tdt)
    iota128 = np.broadcast_to(np.arange(128, dtype=np.float32), (128, 128)).astype(tdt)
    iotaG = np.broadcast_to(np.arange(lay.GPC, dtype=np.float32), (128, lay.GPC)).astype(tdt)
    ident = np.eye(128, dtype=np.float32)
    in_maps = []
    node_col = lay.node_row - 1  # col within the owner core's [D, S] slice
    for c in range(lay.n_cores):
        mask = np.zeros(lay.node_row.shape[0], dtype=bool)
        # nodes owned by core c occupy rows [1+c*S, 1+(c+1)*S)
        mask = (lay.node_row >= 1 + c * lay.S) & (lay.node_row < 1 + (c + 1) * lay.S)
        xTc = np.zeros((D, lay.S), dtype=tdt)
        xTc[:, node_col[mask] - c * lay.S] = x[mask].astype(tdt).T
        m = {
            "xT": xTc,
            "exp0_lo": x_table[lay.slots_lo[c]].reshape(-1, 128, DP),
            "exp0_hi": x_table[lay.slots_hi[c]].reshape(-1, 128, DP),
            "idx_lo": lay.idx_lo[c],
            "idx_hi": lay.idx_hi[c],
            "dstpos": lay.dstpos[c].astype(tdt),
            "graphpos": lay.graphpos[c].astype(tdt),
            "iota128": iota128,
            "iotaG": iotaG,
            "identity": ident,
            "condT": np.ascontiguousarray(cond[lay.graph_lists[c]].T.astype(np.float32)),
        }
        for k, v in wt.items():
            m[k] = np.ascontiguousarray(v)
        in_maps.append(m)
    return in_maps


_CACHE = {}


def _run(inputs, use_bf16=False, trace=False):
    edge_index = np.asarray(inputs["edge_index"])
    batch = np.asarray(inputs["batch"])
    G = int(np.asarray(inputs["cond"]).shape[0])
    key = ("k", edge_index.shape, batch.shape, G, use_bf16)
    if key not in _CACHE:
        lay = build_layout(edge_index, batch, G, n_cores=8, use_bf16=use_bf16)
        nc = build_bass(lay)
        _CACHE[key] = (lay, nc)
    lay, nc = _CACHE[key]
    in_maps = make_in_maps(lay, inputs)
    res = run_bass_kernel_spmd(nc, in_maps, core_ids=list(range(lay.n_cores)),
                               trace=trace)
    G_out = np.zeros((G, 64), dtype=np.float32)
    for c in range(lay.n_cores):
        outT = res.results[c]["outT"]  # [64, GPC]
        G_out[lay.graph_lists[c], :] = outT.T
    return G_out, res


DEFAULT_BF16 = "1"


def kernel(**inputs) -> np.ndarray:
    use_bf16 = os.environ.get("GIN_BF16", DEFAULT_BF16) == "1"
    out, _ = _run(inputs, use_bf16=use_bf16)
    return out



# revision 18
# speedup vs baseline: 1.0408x; 1.0408x over previous
"""Trainium2 Bass kernel for CondGIN (3-layer GIN + graph pooling + cond MLP head).

Strategy (8 NeuronCores, SPMD single NEFF):
  - Graphs are assigned to cores (32 graphs/core, edge-balanced). A core owns its
    graphs' nodes and all edges whose dst lands in them.
  - Node features live in a replicated DRAM "table" [TBL, 128] (256B bf16 rows).
    For layers 1-2, each core dma_gather's h[src] for its edges (merged into
    5-block super-gathers to amortize SWDGE overhead), aggregates per-dst with
    PE matmuls against DVE-built one-hot matrices (exact f32 PSUM, no scatter
    races), runs the GIN MLP feature-major (BN folded into W2/b2 on host),
    writes its slice, and an 8-core AllGather refreshes the table.
  - Layer 0's sources are known on the host (h = x), so they are pre-expanded
    into the gather-slot layout and streamed with sequential DMA — no
    descriptor generation (saves ~1ms of gpsimd SWDGE time).
  - The self contribution h_i is NOT gathered: each block's previous-layer hT
    stays in SBUF (xT / hstoreA / hstoreB ping-pong) and is added into aT
    directly, cutting ~1 gather chunk per block side.
  - Gather indices are int16 (HW limit 32767): the table is addressed via a lo
    base and a hi base aligned to a core boundary; the host block packer
    balances per-block lo/hi edge counts separately so the global chunk counts
    CPB_LO/CPB_HI stay minimal (padding ~2%).
  - Pooling (segment_sum over batch) is a matmul against per-block graph
    one-hots accumulated in PSUM; the tiny cond MLP + FC head run per-core on
    its 32 graphs.

Measured on 8 axon trn2 cores: 2.49 ms HW exec, rel err 2.6e-4 (baseline
dma_gather-everything version: 3.52 ms). Remaining wall: SWDGE descriptor
generation at ~7.75 ns/row on gpsimd for layers 1-2 (~0.8 ms/layer).
"""

import math
import os
from contextlib import ExitStack

import numpy as np

import concourse.bass as bass
import concourse.bacc as bacc
import concourse.mybir as mybir
import concourse.tile as tile
from concourse.bass_utils import run_bass_kernel_spmd

F32 = mybir.dt.float32
BF16 = mybir.dt.bfloat16
I16 = mybir.dt.int16

D = 96          # feature dim
DP = 128        # padded row width (elements)
BN_EPS = 1e-5
LRELU_ALPHA = 0.2


# ----------------------------------------------------------------------------
# Host-side layout construction
# ----------------------------------------------------------------------------

class Layout:
    pass


def build_layout(edge_index, batch, n_graphs, n_cores=8, lo_split=32768,
                 use_bf16=False, block_slack=1):
    """Compute the full static layout + per-core index data from the inputs."""
    lay = Layout()
    src = np.asarray(edge_index[0], dtype=np.int64)
    dst = np.asarray(edge_index[1], dtype=np.int64)
    batch = np.asarray(batch, dtype=np.int64)
    N = batch.shape[0]
    E = src.shape[0]
    G = n_graphs
    lay.n_cores = n_cores
    lay.use_bf16 = use_bf16
    assert G % n_cores == 0
    GPC = G // n_cores
    lay.GPC = GPC

    # graph node ranges (batch is sorted)
    gstart = np.searchsorted(batch, np.arange(G + 1))
    gsize = np.diff(gstart)

    # per-graph edge counts (by dst's graph)
    dst_graph = np.searchsorted(gstart, dst, side="right") - 1
    gedges = np.bincount(dst_graph, minlength=G)

    # graphs -> cores: balanced LPT on (edges + nodes), exactly GPC per core
    order = np.argsort(-(gedges + gsize))
    core_load = np.zeros(n_cores, dtype=np.int64)
    core_cnt = np.zeros(n_cores, dtype=np.int64)
    graph_core = np.zeros(G, dtype=np.int64)
    for g in order:
        open_cores = np.nonzero(core_cnt < GPC)[0]
        c = open_cores[np.argmin(core_load[open_cores])]
        graph_core[g] = c
        core_load[c] += gedges[g] + gsize[g]
        core_cnt[c] += 1
    graph_lists = [np.nonzero(graph_core == c)[0] for c in range(n_cores)]
    lay.graph_lists = graph_lists

    node_core = graph_core[batch]  # core of each node
    # per-node in-degree (real edges)
    indeg = np.bincount(dst, minlength=N)

    # nodes per core
    core_nodes = [np.nonzero(node_core == c)[0] for c in range(n_cores)]
    max_n = max(len(x) for x in core_nodes)
    B = int(math.ceil(max_n / 128.0)) + block_slack
    lay.B = B
    S = B * 128
    lay.S = S
    TBL = 2 + n_cores * S
    lay.TBL = TBL
    # align the int16 lo/hi base split to a core boundary so a node's
    # lo/hi-ness is known before block packing (cores 0..SPLIT_CORE-1 are lo)
    split_core = min((lo_split - 1) // S, n_cores)
    lo_split = 1 + split_core * S
    assert TBL - lo_split <= 32768, (TBL, lo_split)
    assert lo_split <= 32768
    lay.lo_split = lo_split

    # per-node in-degree split by source row half (src core < split_core)
    src_is_lo = node_core[src] < split_core
    indeg_lo = np.bincount(dst[src_is_lo], minlength=N)
    indeg_hi = indeg - indeg_lo

    # --- assign nodes to (block, pos) per core, balancing lo/hi edge counts
    # separately (the gather chunk counts CPB_LO/CPB_HI are global maxima of
    # per-block lo/hi loads, so both must be flat).
    node_block = np.full(N, -1, dtype=np.int64)
    node_pos = np.full(N, -1, dtype=np.int64)
    for c in range(n_cores):
        nodes = core_nodes[c]
        dlo = indeg_lo[nodes]
        dhi = indeg_hi[nodes]
        order = np.argsort(-(dlo + dhi))
        lload = np.zeros(B, dtype=np.int64)
        hload = np.zeros(B, dtype=np.int64)
        bcnt = np.zeros(B, dtype=np.int64)
        avg_l = max(dlo.sum() / B, 1.0)
        avg_h = max(dhi.sum() / B, 1.0)
        for i in order:
            open_b = np.nonzero(bcnt < 128)[0]
            score = np.maximum((lload[open_b] + dlo[i]) / avg_l,
                               (hload[open_b] + dhi[i]) / avg_h)
            b = open_b[np.argmin(score)]
            node_block[nodes[i]] = b
            node_pos[nodes[i]] = bcnt[b]
            lload[b] += dlo[i]
            hload[b] += dhi[i]
            bcnt[b] += 1

    # global table row of each node
    node_row = 1 + node_core * S + node_block * 128 + node_pos
    lay.node_row = node_row

    # --- per (core, block): build edge lists (incl self edges), split lo/hi ---
    # edge entry: (src_row, pos_of_dst)
    all_src_row = node_row[src]
    all_dst_core = node_core[dst]
    all_dst_block = node_block[dst]
    all_dst_pos = node_pos[dst]

    # chunk counts
    lo_cnt = np.zeros((n_cores, B), dtype=np.int64)
    hi_cnt = np.zeros((n_cores, B), dtype=np.int64)
    # bucket edges per (core, block)
    edge_lists = [[None] * B for _ in range(n_cores)]
    key = all_dst_core * B + all_dst_block
    order = np.argsort(key, kind="stable")
    bounds = np.searchsorted(key[order], np.arange(n_cores * B + 1))
    for c in range(n_cores):
        for b in range(B):
            k = c * B + b
            sel = order[bounds[k]:bounds[k + 1]]
            srcs = all_src_row[sel]
            poss = all_dst_pos[sel]
            is_lo = srcs < lo_split
            edge_lists[c][b] = (srcs, poss, is_lo)
            lo_cnt[c, b] = int(is_lo.sum())
            hi_cnt[c, b] = int((~is_lo).sum())

    CPB_LO = int(math.ceil(lo_cnt.max() / 128.0)) if lo_cnt.max() > 0 else 0
    CPB_HI = int(math.ceil(hi_cnt.max() / 128.0)) if hi_cnt.max() > 0 else 0
    CPB_LO = max(CPB_LO, 1)
    lay.CPB_LO, lay.CPB_HI = CPB_LO, CPB_HI
    CPB = CPB_LO + CPB_HI
    lay.CPB = CPB

    zero_lo = 0                      # table row 0 is a zero row
    zero_hi = TBL - 1 - lo_split     # last row is a zero row (hi-relative)

    # --- emit per-core tensors ---
    # absolute source-row ids per gather slot (for host-side l0 pre-expansion)
    slots_lo = np.zeros((n_cores, B * CPB_LO * 128), dtype=np.int64)
    slots_hi = np.full((n_cores, max(B * CPB_HI * 128, 1)), TBL - 1,
                       dtype=np.int64)
    idx_lo = np.zeros((n_cores, 128, B * CPB_LO * 8), dtype=np.int16)
    idx_hi = np.zeros((n_cores, 128, max(B * CPB_HI * 8, 1)), dtype=np.int16)
    fdt = np.float32
    dstpos = np.full((n_cores, 128, B * CPB), -1.0, dtype=fdt)
    graphpos = np.full((n_cores, 128, B), -1.0, dtype=fdt)

    def fill_idx(arr, c, col0, vals):
        # vals: int array of length n; slot i -> group row i%16, col col0+i//16,
        # replicated across the 8 groups of 16 partitions
        n = len(vals)
        cols = int(math.ceil(n / 16.0))
        buf = np.zeros(16 * cols, dtype=np.int16)
        buf[:n] = vals.astype(np.int16)
        buf = buf.reshape(cols, 16).T  # [16, cols]
        for g in range(8):
            arr[c, g * 16:(g + 1) * 16, col0:col0 + cols] = buf

    # gather-merge group size: one dma_gather instruction covers GB blocks
    GB = 5 if B % 5 == 0 else (2 if B % 2 == 0 else 1)
    lay.GB = GB

    for c in range(n_cores):
        vv_acc = {0: [], 1: []}
        for b in range(B):
            srcs, poss, is_lo = edge_lists[c][b]
            for half, (cap, colw, arr, zrow, base) in enumerate([
                (CPB_LO, CPB_LO * 8, idx_lo, zero_lo, 0),
                (CPB_HI, CPB_HI * 8, idx_hi, zero_hi, lo_split),
            ]):
                if cap == 0:
                    continue
                m = is_lo if half == 0 else ~is_lo
                v = srcs[m] - base
                p = poss[m]
                n = len(v)
                assert n <= cap * 128, (c, b, half, n, cap * 128)
                vv = np.full(cap * 128, zrow, dtype=np.int64)
                vv[:n] = v
                vv_acc[half].append(vv)
                slots = slots_lo if half == 0 else slots_hi
                slots[c, b * cap * 128:(b + 1) * cap * 128] = vv + base
                # dstpos: chunk col index within the block = half offset + chunk
                pp = np.full(cap * 128, -1.0, dtype=fdt)
                pp[:n] = p.astype(fdt)
                coff = b * CPB + (0 if half == 0 else CPB_LO)
                dstpos[c, :, coff:coff + cap] = pp.reshape(cap, 128).T
            if (b + 1) % GB == 0:
                # emit the group's concatenated slot lists, 16-wrapped per
                # gather instruction (not per block)
                g0 = b + 1 - GB
                for half, (cap, colw, arr) in enumerate([
                    (CPB_LO, CPB_LO * 8, idx_lo),
                    (CPB_HI, CPB_HI * 8, idx_hi),
                ]):
                    if cap == 0 or not vv_acc[half]:
                        continue
                    fill_idx(arr, c, g0 * colw,
                             np.concatenate(vv_acc[half]))
                vv_acc = {0: [], 1: []}

        # graphpos: slot -> ordinal of its graph within this core's graph list
        gl = graph_lists[c]
        gmap = {g: j for j, g in enumerate(gl)}
        nodes = core_nodes[c]
        for nid in nodes:
            b = node_block[nid]
            p = node_pos[nid]
            graphpos[c, p, b] = float(gmap[batch[nid]])

    lay.idx_lo, lay.idx_hi = idx_lo, idx_hi
    lay.slots_lo, lay.slots_hi = slots_lo, slots_hi
    lay.dstpos, lay.graphpos = dstpos, graphpos
    return lay


def build_x_table(lay, x):
    tdt = np.float32 if not lay.use_bf16 else np.dtype("bfloat16")
    try:
        tdt = np.dtype(tdt)
        tbl = np.zeros((lay.TBL, DP), dtype=tdt)
    except TypeError:
        import ml_dtypes
        tdt = np.dtype(ml_dtypes.bfloat16)
        tbl = np.zeros((lay.TBL, DP), dtype=tdt)
    tbl[lay.node_row, :D] = x.astype(tdt)
    return tbl


def fold_weights(inputs):
    """Fold BN (eval) affine transforms into adjacent linear layers. Returns a
    dict of device-ready weight arrays (float32)."""
    f = {k: np.asarray(v, dtype=np.float64) for k, v in inputs.items()
         if k not in ("x", "cond", "edge_index", "batch")}
    out = {}
    L = f["conv_W1"].shape[0]
    for layer in range(L):
        s = f["conv_g"][layer] / np.sqrt(f["conv_var"][layer] + BN_EPS)
        t = f["conv_beta"][layer] - f["conv_mean"][layer] * s
        W2p = s[:, None] * f["conv_W2"][layer]
        b2p = t @ f["conv_W2"][layer] + f["conv_b2"][layer]
        out[f"W1_{layer}"] = f["conv_W1"][layer].astype(np.float32)          # [96,96] lhsT
        out[f"b1_{layer}"] = f["conv_b1"][layer].astype(np.float32)[:, None]  # [96,1]
        out[f"W2_{layer}"] = W2p.astype(np.float32)
        out[f"b2_{layer}"] = b2p.astype(np.float32)[:, None]
    # cond MLP: bn1 folds into cW1/cb1
    s = f["cg"] / np.sqrt(f["cvar"] + BN_EPS)
    t = f["cbeta"] - f["cmean"] * s
    out["cW1"] = (f["cW1"] * s[None, :]).astype(np.float32)       # [7,5]
    out["cb1"] = ((f["cb1"] * s) + t).astype(np.float32)[:, None]  # [5,1]
    out["cW2"] = f["cW2"].astype(np.float32)                      # [5,5]
    out["cb2"] = f["cb2"].astype(np.float32)[:, None]             # [5,1]
    # final bn folds into fc
    s = f["bn_g"] / np.sqrt(f["bn_var"] + BN_EPS)
    t = f["bn_b"] - f["bn_mean"] * s
    fcW = s[:, None] * f["fc_W"]
    fcb = t @ f["fc_W"] + f["fc_b"]
    CH = f["cW2"].shape[1]
    out["fcWc"] = fcW[:CH].astype(np.float32)        # [5,64]
    out["fcWd"] = fcW[CH:].astype(np.float32)        # [96,64]
    out["fcb"] = fcb.astype(np.float32)[:, None]     # [64,1]
    return out


# ----------------------------------------------------------------------------
# Bass kernel builder
# ----------------------------------------------------------------------------

def build_bass(lay, n_layers=3, lat=64):
    n_cores = lay.n_cores
    B, CPB_LO, CPB_HI, CPB = lay.B, lay.CPB_LO, lay.CPB_HI, lay.CPB
    S, TBL, GPC = lay.S, lay.TBL, lay.GPC
    LO = lay.lo_split
    tdt = BF16 if lay.use_bf16 else F32
    CD = 7
    CH = 5

    nc = bacc.Bacc("TRN2", target_bir_lowering=False, debug=False,
                   num_devices=n_cores)

    # ---- I/O ----
    # layer-0 sources pre-expanded on host into gather-slot layout
    exp0_lo = nc.dram_tensor("exp0_lo", [128, B * CPB_LO, DP], tdt,
                             kind="ExternalInput")
    exp0_hi = nc.dram_tensor("exp0_hi", [128, max(B * CPB_HI, 1), DP], tdt,
                             kind="ExternalInput")
    xT = nc.dram_tensor("xT", [D, S], tdt, kind="ExternalInput")
    idx_lo = nc.dram_tensor("idx_lo", [128, B * CPB_LO * 8], I16, kind="ExternalInput")
    idx_hi = nc.dram_tensor("idx_hi", [128, max(B * CPB_HI * 8, 1)], I16, kind="ExternalInput")
    dstpos = nc.dram_tensor("dstpos", [128, B * CPB], tdt, kind="ExternalInput")
    graphpos = nc.dram_tensor("graphpos", [128, B], tdt, kind="ExternalInput")
    iota128 = nc.dram_tensor("iota128", [128, 128], tdt, kind="ExternalInput")
    iotaG = nc.dram_tensor("iotaG", [128, GPC], tdt, kind="ExternalInput")
    identity = nc.dram_tensor("identity", [128, 128], F32, kind="ExternalInput")
    condT = nc.dram_tensor("condT", [CD, GPC], F32, kind="ExternalInput")
    wnames = []
    for l in range(n_layers):
        wnames += [(f"W1_{l}", [D, D]), (f"b1_{l}", [D, 1]),
                   (f"W2_{l}", [D, D]), (f"b2_{l}", [D, 1])]
    wnames += [("cW1", [CD, CH]), ("cb1", [CH, 1]), ("cW2", [CH, CH]),
               ("cb2", [CH, 1]), ("fcWc", [CH, lat]), ("fcWd", [D, lat]),
               ("fcb", [lat, 1])]
    wt_dram = {nm: nc.dram_tensor(nm, shp, F32, kind="ExternalInput")
               for nm, shp in wnames}
    outT = nc.dram_tensor("outT", [lat, GPC], F32, kind="ExternalOutput")

    with ExitStack() as stack:
        tc = stack.enter_context(tile.TileContext(nc))

        dram = stack.enter_context(tc.tile_pool(name="dram", bufs=1, space="DRAM"))
        table_int = dram.tile([TBL, DP], tdt)
        my_slice = dram.tile([S, DP], tdt)

        const = stack.enter_context(tc.tile_pool(name="const", bufs=1))
        sb = {}
        for nm, shp in wnames:
            sb[nm] = const.tile(shp, F32, name=f"sb_{nm}")
            nc.sync.dma_start(sb[nm], wt_dram[nm].ap())
        idx_lo_sb = const.tile([128, B * CPB_LO * 8], I16, name="idx_lo_sb")
        nc.sync.dma_start(idx_lo_sb, idx_lo.ap())
        if CPB_HI > 0:
            idx_hi_sb = const.tile([128, B * CPB_HI * 8], I16, name="idx_hi_sb")
            nc.sync.dma_start(idx_hi_sb, idx_hi.ap())
        dstpos_sb = const.tile([128, B * CPB], tdt, name="dstpos_sb")
        nc.sync.dma_start(dstpos_sb, dstpos.ap())
        graphpos_sb = const.tile([128, B], tdt, name="graphpos_sb")
        nc.sync.dma_start(graphpos_sb, graphpos.ap())
        iota128_sb = const.tile([128, 128], tdt, name="iota128_sb")
        nc.sync.dma_start(iota128_sb, iota128.ap())
        iotaG_sb = const.tile([128, GPC], tdt, name="iotaG_sb")
        nc.sync.dma_start(iotaG_sb, iotaG.ap())
        ident_sb = const.tile([128, 128], F32, name="ident_sb")
        nc.sync.dma_start(ident_sb, identity.ap())
        condT_sb = const.tile([CD, GPC], F32, name="condT_sb")
        nc.sync.dma_start(condT_sb, condT.ap())
        xT_sb = const.tile([D, S], tdt, name="xT_sb")
        nc.sync.dma_start(xT_sb, xT.ap())
        hstoreA = const.tile([D, S], tdt, name="hstoreA")
        hstoreB = const.tile([D, S], tdt, name="hstoreB")
        zero_sb = const.tile([1, DP], tdt, name="zero_sb")
        nc.vector.memset(zero_sb, 0.0)
        # zero rows of the internal table (rows 0 and TBL-1; AG writes 1..TBL-2)
        nc.sync.dma_start(table_int[0:1, :], zero_sb)
        nc.sync.dma_start(table_int[TBL - 1:TBL, :], zero_sb)

        # pools
        glo_p = stack.enter_context(tc.tile_pool(name="glo", bufs=3))
        ghi_p = stack.enter_context(tc.tile_pool(name="ghi", bufs=3))
        oh_p = stack.enter_context(tc.tile_pool(name="oh", bufs=3))
        mlp_p = stack.enter_context(tc.tile_pool(name="mlp", bufs=3))
        rows_p = stack.enter_context(tc.tile_pool(name="rows", bufs=3))
        psa_p = stack.enter_context(tc.tile_pool(name="psa", bufs=2, space="PSUM"))
        psm_p = stack.enter_context(tc.tile_pool(name="psm", bufs=4, space="PSUM"))
        psp_p = stack.enter_context(tc.tile_pool(name="psp", bufs=1, space="PSUM"))

        pooled_ps = None
        nreg_lo = CPB_LO * 128
        nreg_hi = CPB_HI * 128

        GB = lay.GB
        NGRP = B // GB
        nsup_lo = GB * CPB_LO * 128
        nsup_hi = GB * CPB_HI * 128
        lo_ap = table_int[0:LO, :]
        hi_ap = table_int[LO:TBL, :] if TBL > LO else None

        for l in range(n_layers):
            last = l == n_layers - 1
            if last:
                pooled_ps = psp_p.tile([D, GPC], F32, name="pooled_ps")

            for grp in range(NGRP):
              b0 = grp * GB
              if True:
                glo = glo_p.tile([128, GB * CPB_LO, DP], tdt, name="glo", tag="glo")
                if l == 0:
                    nc.sync.dma_start(
                        glo,
                        exp0_lo.ap()[:, b0 * CPB_LO:(b0 + GB) * CPB_LO])
                else:
                    nc.gpsimd.dma_gather(
                        glo, lo_ap,
                        idx_lo_sb[:, b0 * CPB_LO * 8:(b0 + GB) * CPB_LO * 8],
                        nsup_lo, nsup_lo, DP, single_packet=False)
                if CPB_HI > 0:
                    ghi = ghi_p.tile([128, GB * CPB_HI, DP], tdt, name="ghi", tag="ghi")
                    if l == 0:
                        nc.sync.dma_start(
                            ghi,
                            exp0_hi.ap()[:, b0 * CPB_HI:(b0 + GB) * CPB_HI])
                    else:
                        nc.gpsimd.dma_gather(
                            ghi, hi_ap,
                            idx_hi_sb[:, b0 * CPB_HI * 8:(b0 + GB) * CPB_HI * 8],
                            nsup_hi, nsup_hi, DP, single_packet=False)

              for bb in range(GB):
                b = b0 + bb
                # one-hot for all chunks of this block: [128, CPB, 128]
                oh = oh_p.tile([128, CPB, 128], tdt, name="oh", tag="oh")
                iota_b = bass.AP(iota128_sb.tensor, iota128_sb.offset,
                                 [iota128_sb.ap[0], [0, CPB], [1, 128]])
                dp_b = dstpos_sb[:, b * CPB:(b + 1) * CPB]
                dp_bb = bass.AP(dp_b.tensor, dp_b.offset,
                                [dp_b.ap[0], [1, CPB], [0, 128]])
                nc.vector.tensor_tensor(out=oh, in0=dp_bb, in1=iota_b,
                                        op=mybir.AluOpType.is_equal)

                # aggregate: psum[dp_feat, pos] += gathered.T @ onehot
                ps_a = psa_p.tile([DP, 128], F32, name="ps_a", tag="ps_a")
                for c in range(CPB):
                    g = (glo[:, bb * CPB_LO + c] if c < CPB_LO
                         else ghi[:, bb * CPB_HI + (c - CPB_LO)])
                    nc.tensor.matmul(ps_a, g, oh[:, c],
                                     start=(c == 0), stop=(c == CPB - 1))

                # MLP (feature-major); add h_prev (self edge done locally)
                src_h = [xT_sb, hstoreA, hstoreB][l]
                aT = mlp_p.tile([D, 128], F32, name="aT", tag="aT")
                nc.vector.tensor_tensor(out=aT, in0=ps_a[0:D, :],
                                        in1=src_h[:, b * 128:(b + 1) * 128],
                                        op=mybir.AluOpType.add)
                def lrelu_bias(out_tile, ps, bias_ap, tagpfx):
                    y = mlp_p.tile([D, 128], F32, name=f"{tagpfx}y",
                                   tag=f"{tagpfx}y")
                    t = mlp_p.tile([D, 128], F32, name=f"{tagpfx}t",
                                   tag=f"{tagpfx}t")
                    nc.vector.tensor_scalar_add(y, ps, bias_ap)
                    nc.vector.tensor_scalar(t, ps, bias_ap, LRELU_ALPHA,
                                            op0=mybir.AluOpType.add,
                                            op1=mybir.AluOpType.mult)
                    nc.vector.tensor_tensor(out=out_tile, in0=y, in1=t,
                                            op=mybir.AluOpType.max)

                ps1 = psm_p.tile([D, 128], F32, name="ps1", tag="psm")
                nc.tensor.matmul(ps1, sb[f"W1_{l}"], aT, start=True, stop=True)
                u = mlp_p.tile([D, 128], F32, name="u", tag="u")
                lrelu_bias(u, ps1, sb[f"b1_{l}"], "u")
                ps2 = psm_p.tile([D, 128], F32, name="ps2", tag="psm")
                nc.tensor.matmul(ps2, sb[f"W2_{l}"], u, start=True, stop=True)
                hT = mlp_p.tile([D, 128], F32, name="hT", tag="hT")
                lrelu_bias(hT, ps2, sb[f"b2_{l}"], "h")
                if not last:
                    dst_h = [hstoreA, hstoreB, None][l]
                    nc.vector.tensor_copy(dst_h[:, b * 128:(b + 1) * 128], hT)
                # transpose to rows
                ps3 = psm_p.tile([128, D], F32, name="ps3", tag="psm")
                nc.tensor.transpose(ps3, hT, ident_sb[0:D, 0:D])
                hrows = rows_p.tile([128, DP], tdt, name="hrows", tag="hrows")
                nc.vector.tensor_copy(hrows[:, 0:D], ps3)
                if not last:
                    nc.vector.memset(hrows[:, D:DP], 0.0)
                    nc.sync.dma_start(my_slice[b * 128:(b + 1) * 128, :], hrows)
                else:
                    ohg = mlp_p.tile([128, GPC], tdt, name="ohg", tag="ohg")
                    gp_b = graphpos_sb[:, b:b + 1]
                    gp_bb = bass.AP(gp_b.tensor, gp_b.offset,
                                    [gp_b.ap[0], [0, GPC]])
                    nc.vector.tensor_tensor(out=ohg, in0=iotaG_sb, in1=gp_bb,
                                            op=mybir.AluOpType.is_equal)
                    nc.tensor.matmul(pooled_ps, hrows[:, 0:D], ohg,
                                     start=(b == 0), stop=(b == B - 1),
                                     skip_group_check=True)

            if not last:
                nc.gpsimd.collective_compute(
                    "AllGather", mybir.AluOpType.bypass,
                    replica_groups=[list(range(n_cores))],
                    ins=[my_slice.opt()],
                    outs=[table_int[1:TBL - 1, :].opt()],
                )

        # ---- head ----
        pooled_sb = const.tile([D, GPC], F32, name="pooled_sb")
        nc.vector.tensor_copy(pooled_sb, pooled_ps)
        psc = psm_p.tile([CH, GPC], F32, name="psc", tag="psm")
        nc.tensor.matmul(psc, sb["cW1"], condT_sb, start=True, stop=True)
        c1 = const.tile([CH, GPC], F32, name="c1")
        nc.scalar.activation(c1, psc, mybir.ActivationFunctionType.Relu,
                             bias=sb["cb1"], scale=1.0)
        psc2 = psm_p.tile([CH, GPC], F32, name="psc2", tag="psm")
        nc.tensor.matmul(psc2, sb["cW2"], c1, start=True, stop=True)
        c2 = const.tile([CH, GPC], F32, name="c2")
        nc.scalar.activation(c2, psc2, mybir.ActivationFunctionType.Relu,
                             bias=sb["cb2"], scale=1.0)
        pso = psm_p.tile([lat, GPC], F32, name="pso", tag="psm")
        nc.tensor.matmul(pso, sb["fcWc"], c2, start=True, stop=False)
        nc.tensor.matmul(pso, sb["fcWd"], pooled_sb, start=False, stop=True)
        out_sb = const.tile([lat, GPC], F32, name="out_sb")
        nc.vector.tensor_scalar_add(out_sb, pso, sb["fcb"])
        nc.sync.dma_start(outT.ap(), out_sb)

    nc.compile()
    return nc


# ----------------------------------------------------------------------------
# Entry point
# ----------------------------------------------------------------------------

def _np_bf16():
    import ml_dtypes
    return np.dtype(ml_dtypes.bfloat16)


def make_in_maps(lay, inputs, n_layers=3, lat=64):
    x = np.asarray(inputs["x"], dtype=np.float32)
    cond = np.asarray(inputs["cond"], dtype=np.float32)
    tdt = _np_bf16() if lay.use_bf16 else np.float32
    wt = fold_weights(inputs)
    x_table = np.zeros((lay.TBL, DP), dtype=tdt)
    x_table[lay.node_row, :D] = x.astype(tdt)
    iota128 = np.broadcast_to(np.arange(128, dtype=np.float32), (128, 128)).astype(tdt)
    iotaG = np.broadcast_to(np.arange(lay.GPC, dtype=np.float32), (128, lay.GPC)).astype(tdt)
    ident = np.eye(128, dtype=np.float32)
    in_maps = []
    node_col = lay.node_row - 1  # col within the owner core's [D, S] slice
    for c in range(lay.n_cores):
        mask = np.zeros(lay.node_row.shape[0], dtype=bool)
        # nodes owned by core c occupy rows [1+c*S, 1+(c+1)*S)
        mask = (lay.node_row >= 1 + c * lay.S) & (lay.node_row < 1 + (c + 1) * lay.S)
        xTc = np.zeros((D, lay.S), dtype=tdt)
        xTc[:, node_col[mask] - c * lay.S] = x[mask].astype(tdt).T
        m = {
            "xT": xTc,
            "exp0_lo": np.ascontiguousarray(
                x_table[lay.slots_lo[c]].reshape(-1, 128, DP).swapaxes(0, 1)),
            "exp0_hi": np.ascontiguousarray(
                x_table[lay.slots_hi[c]].reshape(-1, 128, DP).swapaxes(0, 1)),
            "idx_lo": lay.idx_lo[c],
            "idx_hi": lay.idx_hi[c],
            "dstpos": lay.dstpos[c].astype(tdt),
            "graphpos": lay.graphpos[c].astype(tdt),
            "iota128": iota128,
            "iotaG": iotaG,
            "identity": ident,
            "condT": np.ascontiguousarray(cond[lay.graph_lists[c]].T.astype(np.float32)),
        }
        for k, v in wt.items():
            m[k] = np.ascontiguousarray(v)
        in_maps.append(m)
    return in_maps


_CACHE = {}


def _run(inputs, use_bf16=False, trace=False):
    edge_index = np.asarray(inputs["edge_index"])
    batch = np.asarray(inputs["batch"])
    G = int(np.asarray(inputs["cond"]).shape[0])
    key = ("k", edge_index.shape, batch.shape, G, use_bf16)
    if key not in _CACHE:
        lay = build_layout(edge_index, batch, G, n_cores=8, use_bf16=use_bf16)
        nc = build_bass(lay)
        _CACHE[key] = (lay, nc)
    lay, nc = _CACHE[key]
    in_maps = make_in_maps(lay, inputs)
    res = run_bass_kernel_spmd(nc, in_maps, core_ids=list(range(lay.n_cores)),
                               trace=trace)
    G_out = np.zeros((G, 64), dtype=np.float32)
    for c in range(lay.n_cores):
        outT = res.results[c]["outT"]  # [64, GPC]
        G_out[lay.graph_lists[c], :] = outT.T
    return G_out, res


DEFAULT_BF16 = "1"


def kernel(**inputs) -> np.ndarray:
    use_bf16 = os.environ.get("GIN_BF16", DEFAULT_BF16) == "1"
    out, _ = _run(inputs, use_bf16=use_bf16)
    return out

